# revision 15
# baseline (speedup 1.0000x reference)
"""Multi-head causal self-attention (B=2, N=4096, C=512, H=8, D=64) on 8 TRN2 cores.

Sharding: core = b*4 + g  (b = batch 0..1, g = head-group 0..3, 2 heads each).
Each core computes qkv^T for its 2 heads from x[b]^T, causal attention, and a
partial output projection over its 128 channels.  Host sums the 4 partial y^T
per batch and adds the bias.

Attention structure per (qt=512 queries, head):
  - S units: psS[128 keys, 512*w] = K_blk^T-stationary matmuls (kb groups w<=2)
  - exp on the Activation engine (psS -> Pf bf16 in SBUF)
  - causal mask on the diagonal 128x128 blocks via DVE multiply with `tri`
  - AV transposed: for each (kb, 128-query block qq):
        psO[h][:, qq, 0:65] += Pf[:, kb, qq-slice].T @ [V_kb | ones]
    so the output free dim is 65 (64 dims + softmax denominator), 8x cheaper
    on the PE than streaming 512 queries, and the denominator lands
    per-PARTITION (per query) for a cheap DVE normalize.
  - epilogue: rc = 1/psO[:,qq,64]; ot[q, 64h:64h+64] = psO[:,qq,0:64]*rc (DVE)
  - PE transpose ot[128q, 128d] -> on[128d, 128q]; proj psY = wp^T @ on;
    GPSIMD copies psY -> SBUF; DMA to yt.

The whole thing is software-pipelined with a deferral heap: AV matmuls of unit
i are emitted after the S+exp of unit i+2; epilogue/transpose/proj trail
further to stay off the PE/ACT critical path.
"""

import os

import numpy as np
import ml_dtypes

_CACHE: dict = {}
LAST_RESULTS = None

B, C = 2, 512
H, D = 8, 64
N = 4096
NQT = 8          # q tiles of 512
NKB = 32         # key blocks of 128
QT = 512
KB = 128

# Every EXPM-th exp unit runs entirely on the DVE via a Schraudolph bit-trick
# (bf16 bit pattern built with an affine op into an int16 tile); the rest run
# on the ACT engine.  0 disables DVE offload.
EXPM = int(os.environ.get("EXPM", "4"))
# bf16 Schraudolph constants: bits16 = round(x * 128/ln2 + (127*128 + sigma))
SCHRAUD_A = 184.6649652337873
SCHRAUD_B = 16256.0 - 7.9


def _build():
    import concourse.bass as bass
    import concourse.bacc as bacc
    import concourse.mybir as mybir
    import concourse.tile as tile

    dt = mybir.dt
    bf = dt.bfloat16
    f32 = dt.float32
    i16 = dt.int16
    Exp = mybir.ActivationFunctionType.Exp
    Mult = mybir.AluOpType.mult

    nc = bacc.Bacc("TRN2", target_bir_lowering=False)
    xt = nc.dram_tensor("xt", [C, N], bf, kind="ExternalInput")
    wq = nc.dram_tensor("wq", [C, 128], bf, kind="ExternalInput")
    wk = nc.dram_tensor("wk", [C, 128], bf, kind="ExternalInput")
    wv = nc.dram_tensor("wv", [C, 128], bf, kind="ExternalInput")
    wp = nc.dram_tensor("wp", [128, C], bf, kind="ExternalInput")
    tri = nc.dram_tensor("tri", [128, 128], bf, kind="ExternalInput")
    yt = nc.dram_tensor("yt", [C, N], f32, kind="ExternalOutput")

    with tile.TileContext(nc) as tc:
        with (
            tc.tile_pool(name="persist", bufs=1) as pp,
            tc.tile_pool(name="pf", bufs=5) as pf_pool,
            tc.tile_pool(name="pd", bufs=3) as pd_pool,
            tc.tile_pool(name="ot", bufs=8) as ot_pool,
            tc.tile_pool(name="on", bufs=2) as on_pool,
            tc.tile_pool(name="rc", bufs=4) as rc_pool,
            tc.tile_pool(name="yo", bufs=3) as yo_pool,
            tc.tile_pool(name="ps_s", bufs=3, space="PSUM") as ps_s,
            tc.tile_pool(name="ps_p", bufs=1, space="PSUM") as ps_p,
        ):
            xt_sb = pp.tile([128, 4, N], bf)
            wq_sb = pp.tile([128, 4, 128], bf)
            wk_sb = pp.tile([128, 4, 128], bf)
            wv_sb = pp.tile([128, 4, 128], bf)
            wp_sb = pp.tile([128, C], bf)
            tri_sb = pp.tile([128, 128], bf)
            qT = pp.tile([128, N], bf)
            kT = pp.tile([128, N], bf)
            v_sb = pp.tile([128, NKB, 130], bf)

            # Per-head AV accumulators: head h owns one full PSUM bank
            # (4 x 65 slots at [512h : 512h+260]).  matmul start=True clears
            # the has_written bits of the WHOLE 2KB bank (data is untouched),
            # so each head's row places exactly one start=True (kb==0, qq==0)
            # and the two heads' rows, which are interleaved in time, never
            # disturb each other.
            psPm = ps_p.tile([128, 1024], f32, name="psPm")

            def psO_ap(h, qq):
                return psPm[:, 512 * h + 65 * qq:512 * h + 65 * qq + 65]

            nc.gpsimd.dma_start(out=wq_sb[:, :, :], in_=wq.rearrange("(c p) f -> p c f", p=128))
            nc.gpsimd.dma_start(out=wk_sb[:, :, :], in_=wk.rearrange("(c p) f -> p c f", p=128))
            nc.gpsimd.dma_start(out=wv_sb[:, :, :], in_=wv.rearrange("(c p) f -> p c f", p=128))
            nc.gpsimd.dma_start(out=wp_sb, in_=wp[:, :])
            nc.gpsimd.dma_start(out=tri_sb, in_=tri[:, :])
            nc.vector.memset(v_sb, 1.0)

            xt_re = xt.rearrange("(c p) n -> p c n", p=128)

            exp_counter = [0]

            def emit_exp(dst_i16, src_ps, cols, dve_ok=True):
                """exp(src_ps[:, 0:cols]) -> bf16 bit patterns in dst_i16,
                on ACT normally, on DVE (Schraudolph affine bit-trick) for
                every EXPM-th unit (unless dve_ok is False)."""
                exp_counter[0] += 1
                if dve_ok and EXPM > 0 and exp_counter[0] % EXPM == 0:
                    nc.vector.tensor_scalar(
                        out=dst_i16[:, 0:cols],
                        in0=src_ps[:, 0:cols],
                        scalar1=SCHRAUD_A,
                        scalar2=SCHRAUD_B,
                        op0=Mult,
                        op1=mybir.AluOpType.add,
                    )
                else:
                    nc.scalar.activation(
                        dst_i16.bitcast(bf)[:, 0:cols], src_ps[:, 0:cols], Exp
                    )

            def pa_qk(n, dst, wsb, with_dma):
                def piece():
                    if with_dma:
                        nc.sync.dma_start(
                            out=xt_sb[:, :, QT * n:QT * (n + 1)],
                            in_=xt_re[:, :, QT * n:QT * (n + 1)],
                        )
                    ps = ps_s.tile([128, 1024], f32, tag="s", name=f"pa_{n}")
                    for c in range(4):
                        nc.tensor.matmul(
                            ps[:, 0:512],
                            wsb[:, c, :],
                            xt_sb[:, c, QT * n:QT * (n + 1)],
                            start=(c == 0),
                            stop=(c == 3),
                        )
                    nc.vector.tensor_copy(dst[:, QT * n:QT * (n + 1)], ps[:, 0:512])
                return piece

            def pa_v(kb):
                def piece():
                    ps = ps_s.tile([128, 1024], f32, tag="s", name=f"pav_{kb}")
                    pv = ps[:, 0:128]
                    for c in range(4):
                        nc.tensor.matmul(
                            pv,
                            xt_sb[:, c, KB * kb:KB * (kb + 1)],
                            wv_sb[:, c, :],
                            start=(c == 0),
                            stop=(c == 3),
                        )
                    nc.vector.tensor_copy(
                        v_sb[:, kb, :].rearrange("p (h j) -> p h j", h=2)[:, :, 0:64],
                        pv.rearrange("p (h j) -> p h j", h=2),
                    )
                return piece

            def phase_a_pieces(n):
                return [
                    pa_qk(n, qT, wq_sb, True),
                    pa_qk(n, kT, wk_sb, False),
                    pa_v(4 * n),
                    pa_v(4 * n + 1),
                    pa_v(4 * n + 2),
                    pa_v(4 * n + 3),
                ]

            # diag slot layout keeps every matmul inside one 2KB PSUM bank:
            # r1 -> [0:384], r3 -> [384:512] (bank 0), r2 -> [512:768] (bank 1)
            offs = (0, 512, 384)
            wid = (384, 256, 128)
            # Pd column holding the 128-wide slice for (r, qq):
            # r=1: qq 1,2,3 at 0,128,256 ; r=2: qq 2,3 at 512,640 ; r=3: qq 3 at 384
            diag_col = {(1, 1): 0, (1, 2): 128, (1, 3): 256,
                        (2, 2): 512, (2, 3): 640, (3, 3): 384}

            ot_map = {}
            on_map = {}
            import heapq
            deferred = []  # heap of (due_unit_index, seq, closure)
            seq_counter = [0]

            def defer(due, fn):
                heapq.heappush(deferred, (due, seq_counter[0], fn))
                seq_counter[0] += 1

            def flush(i):
                while deferred and deferred[0][0] <= i:
                    heapq.heappop(deferred)[2]()

            def make_av_full(qt, h, kbs, Pf):
                def av():
                    for j, kb in enumerate(kbs):
                        for qq in range(4):
                            nc.tensor.matmul(
                                psO_ap(h, qq),
                                Pf[:, 512 * j + 128 * qq:512 * j + 128 * (qq + 1)],
                                v_sb[:, kb, 65 * h:65 * h + 65],
                                start=(kb == 0 and qq == 0),
                                stop=(kb == 4 * qt and qq == 0),
                                skip_group_check=True,
                            )
                return av

            def make_av_diag(qt, h, Pd):
                def av():
                    for r in (1, 2, 3):
                        kb = 4 * qt + r
                        for qq in range(r, 4):
                            col = diag_col[(r, qq)]
                            nc.tensor.matmul(
                                psO_ap(h, qq),
                                Pd[:, col:col + 128],
                                v_sb[:, kb, 65 * h:65 * h + 65],
                                start=False,
                                stop=(r == qq),
                                skip_group_check=True,
                            )
                return av

            def make_epilogue(qt, h):
                # Batched normalize: one reciprocal + one multiply over all of
                # this head's slots (strided [65|130] views of psPm), writing
                # the packed [q, d] tile ot_map[qt][:, 128*qq+64*h : +64].
                def epi():
                    if qt not in ot_map:
                        ot_map[qt] = ot_pool.tile(
                            [128, 512], bf, tag="ot", name=f"ot_{qt}"
                        )
                    ot = ot_map[qt].rearrange("p (a b j) -> p a b j", a=4, b=2)
                    rc = rc_pool.tile([128, 4], f32, tag="rc")
                    v4 = psPm[:, 512 * h:512 * h + 260].rearrange(
                        "p (s j) -> p s j", s=4
                    )
                    nc.vector.reciprocal(out=rc, in_=v4[:, :, 64:65])
                    nc.vector.tensor_mul(
                        ot[:, :, h, :],
                        v4[:, :, 0:64],
                        rc.unsqueeze(-1).broadcast_to((128, 4, 64)),
                    )
                return epi

            def make_transposes(qt, qqs):
                def tp():
                    if qt not in on_map:
                        on_map[qt] = on_pool.tile([128, 512], bf, tag="on", name=f"on_{qt}")
                    on = on_map[qt]
                    ot = ot_map[qt]
                    for qq in qqs:
                        nc.sync.dma_start_transpose(
                            on[:, 128 * qq:128 * (qq + 1)],
                            ot[:, 128 * qq:128 * (qq + 1)],
                        )
                return tp

            def make_proj_ob(qt, ob):
                def proj():
                    on = on_map[qt]
                    psYt = ps_s.tile([128, 1024], f32, tag="s", name=f"psY_{qt}_{ob}")
                    psY = psYt[:, 0:512]
                    for qq in range(4):
                        nc.tensor.matmul(
                            psY[:, 128 * qq:128 * (qq + 1)],
                            wp_sb[:, 128 * ob:128 * (ob + 1)],
                            on[:, 128 * qq:128 * (qq + 1)],
                            start=True,
                            stop=True,
                        )
                    y_sb = yo_pool.tile([128, 512], f32, tag="yo")
                    nc.vector.tensor_copy(y_sb, psY)
                    nc.sync.dma_start(
                        out=yt[128 * ob:128 * (ob + 1), QT * qt:QT * (qt + 1)],
                        in_=y_sb,
                    )
                    if ob == 3:
                        on_map.pop(qt)
                        ot_map.pop(qt)
                return proj

            ui = 0
            full_unit_idx = [0]
            pa_qk(0, qT, wq_sb, True)()
            pa_qk(0, kT, wk_sb, False)()
            pa_pending = [pa_v(0), pa_v(1), pa_v(2), pa_v(3)]
            for qt in range(NQT):
                for piece in pa_pending:
                    piece()
                pa_pending = phase_a_pieces(qt + 1) if qt + 1 < NQT else []
                nfull = 4 * qt + 1
                d_av = 2 if nfull <= 4 else 3
                # ---- full units: kb groups of 2, heads interleaved per group
                kb = 0
                while kb < nfull:
                    w = min(2, nfull - kb)
                    kbs = list(range(kb, kb + w))
                    for h in range(2):
                        b0 = 64 * h
                        psS = ps_s.tile([128, 1024], f32, tag="s")
                        for j, kbj in enumerate(kbs):
                            nc.tensor.matmul(
                                psS[:, 512 * j:512 * (j + 1)],
                                kT[b0:b0 + 64, KB * kbj:KB * (kbj + 1)],
                                qT[b0:b0 + 64, QT * qt:QT * (qt + 1)],
                                start=True,
                                stop=True,
                            )
                        pfi = pf_pool.tile([128, 1024], i16, tag="pf")
                        emit_exp(pfi, psS, 512 * w, dve_ok=(kb + w < nfull - 2))
                        Pf = pfi.bitcast(bf)
                        if kbs[-1] == 4 * qt:
                            j = w - 1
                            nc.gpsimd.tensor_mul(
                                Pf[:, 512 * j:512 * j + 128],
                                Pf[:, 512 * j:512 * j + 128],
                                tri_sb,
                            )
                        flush(ui)
                        defer(ui + d_av, make_av_full(qt, h, kbs, Pf))
                        if pa_pending:
                            pa_pending.pop(0)()
                        ui += 1
                    kb += w
                # ---- diag units: r = 1..3 packed [r1|r3|r2], heads paired
                for h in range(2):
                    b0 = 64 * h
                    psD = ps_s.tile([128, 1024], f32, tag="s")
                    for r in (1, 2, 3):
                        kbr = 4 * qt + r
                        nc.tensor.matmul(
                            psD[:, offs[r - 1]:offs[r - 1] + wid[r - 1]],
                            kT[b0:b0 + 64, KB * kbr:KB * (kbr + 1)],
                            qT[b0:b0 + 64, QT * qt + 128 * r:QT * qt + 128 * r + wid[r - 1]],
                            start=True,
                            stop=True,
                        )
                    Pdi = pd_pool.tile([128, 768], i16, tag="pd")
                    emit_exp(Pdi, psD, 768, dve_ok=False)
                    Pd = Pdi.bitcast(bf)
                    for r in (1, 2, 3):
                        nc.gpsimd.tensor_mul(
                            Pd[:, offs[r - 1]:offs[r - 1] + 128],
                            Pd[:, offs[r - 1]:offs[r - 1] + 128],
                            tri_sb,
                        )
                    flush(ui)
                    defer(ui + d_av, make_av_diag(qt, h, Pd))
                    defer(ui + d_av + 1, make_epilogue(qt, h))
                    if h == 1:
                        defer(ui + d_av + 1, make_transposes(qt, (0, 1)))
                        defer(ui + d_av + 2, make_transposes(qt, (2, 3)))
                        for ob in range(4):
                            defer(ui + d_av + 3 + ob, make_proj_ob(qt, ob))
                    if pa_pending:
                        pa_pending.pop(0)()
                    ui += 1
            flush(10 ** 9)

    nc.compile()
    return nc


def kernel(x, w_qkv, w_proj, b_proj):
    global LAST_RESULTS
    from concourse.bass_utils import run_bass_kernel_spmd

    if "nc" not in _CACHE:
        _CACHE["nc"] = _build()
    nc = _CACHE["nc"]

    x = np.asarray(x)
    w_qkv = np.asarray(w_qkv)
    w_proj = np.asarray(w_proj)
    b_proj = np.asarray(b_proj)
    bf16 = ml_dtypes.bfloat16
    scale = D ** -0.5

    tri = np.triu(np.ones((128, 128), np.float32)).astype(bf16)
    in_maps = []
    for core in range(8):
        b, g = divmod(core, 4)
        xt = np.ascontiguousarray(x[b].T).astype(bf16)
        wq = np.ascontiguousarray((w_qkv[128 * g:128 * (g + 1), :].T * scale)).astype(bf16)
        wk = np.ascontiguousarray(w_qkv[C + 128 * g:C + 128 * (g + 1), :].T).astype(bf16)
        wv = np.ascontiguousarray(w_qkv[2 * C + 128 * g:2 * C + 128 * (g + 1), :].T).astype(bf16)
        wp = np.ascontiguousarray(w_proj[:, 128 * g:128 * (g + 1)].T).astype(bf16)
        in_maps.append({"xt": xt, "wq": wq, "wk": wk, "wv": wv, "wp": wp,
                        "tri": tri})

    res = run_bass_kernel_spmd(
        nc,
        in_maps,
        core_ids=list(range(8)),
        trace=bool(os.environ.get("KERNEL_TRACE")),
    )
    LAST_RESULTS = res

    y = np.empty((B, N, C), np.float32)
    for b in range(B):
        acc = res.results[4 * b]["yt"].astype(np.float32)
        for g in range(1, 4):
            acc = acc + res.results[4 * b + g]["yt"]
        y[b] = acc.T + b_proj
    return y


# revision 16
# speedup vs baseline: 1.0723x; 1.0723x over previous
"""Multi-head causal self-attention (B=2, N=4096, C=512, H=8, D=64) on 8 TRN2 cores.

Sharding: core = b*4 + g  (b = batch 0..1, g = head-group 0..3, 2 heads each).
Each core computes qkv^T for its 2 heads from x[b]^T, causal attention, and a
partial output projection over its 128 channels.  Host sums the 4 partial y^T
per batch and adds the bias.

Attention structure per (qt=512 queries, head):
  - S units: psS[128 keys, 512*w] = K_blk^T-stationary matmuls (kb groups w<=2)
  - exp on the Activation engine (psS -> Pf bf16 in SBUF)
  - causal mask on the diagonal 128x128 blocks via DVE multiply with `tri`
  - AV transposed: for each (kb, 128-query block qq):
        psO[h][:, qq, 0:65] += Pf[:, kb, qq-slice].T @ [V_kb | ones]
    so the output free dim is 65 (64 dims + softmax denominator), 8x cheaper
    on the PE than streaming 512 queries, and the denominator lands
    per-PARTITION (per query) for a cheap DVE normalize.
  - epilogue: rc = 1/psO[:,qq,64]; ot[q, 64h:64h+64] = psO[:,qq,0:64]*rc (DVE)
  - PE transpose ot[128q, 128d] -> on[128d, 128q]; proj psY = wp^T @ on;
    GPSIMD copies psY -> SBUF; DMA to yt.

The whole thing is software-pipelined with a deferral heap: AV matmuls of unit
i are emitted after the S+exp of unit i+2; epilogue/transpose/proj trail
further to stay off the PE/ACT critical path.
"""

import os

import numpy as np
import ml_dtypes

_CACHE: dict = {}
LAST_RESULTS = None

B, C = 2, 512
H, D = 8, 64
N = 4096
NQT = 8          # q tiles of 512
NKB = 32         # key blocks of 128
QT = 512
KB = 128

# Every EXPM-th exp unit runs entirely on the DVE via a Schraudolph bit-trick
# (bf16 bit pattern built with an affine op into an int16 tile); the rest run
# on the ACT engine.  0 disables DVE offload.
EXPM = int(os.environ.get("EXPM", "4"))
# bf16 Schraudolph constants: bits16 = round(x * 128/ln2 + (127*128 + sigma))
SCHRAUD_A = 184.6649652337873
SCHRAUD_B = 16256.0 - 7.9


def _build():
    import concourse.bass as bass
    import concourse.bacc as bacc
    import concourse.mybir as mybir
    import concourse.tile as tile

    dt = mybir.dt
    bf = dt.bfloat16
    f32 = dt.float32
    i16 = dt.int16
    Exp = mybir.ActivationFunctionType.Exp
    Mult = mybir.AluOpType.mult

    nc = bacc.Bacc("TRN2", target_bir_lowering=False)
    xt = nc.dram_tensor("xt", [C, N], bf, kind="ExternalInput")
    wq = nc.dram_tensor("wq", [C, 128], bf, kind="ExternalInput")
    wk = nc.dram_tensor("wk", [C, 128], bf, kind="ExternalInput")
    wv = nc.dram_tensor("wv", [C, 128], bf, kind="ExternalInput")
    wp = nc.dram_tensor("wp", [128, C], bf, kind="ExternalInput")
    tri = nc.dram_tensor("tri", [128, 128], bf, kind="ExternalInput")
    yt = nc.dram_tensor("yt", [C, N], f32, kind="ExternalOutput")

    with tile.TileContext(nc) as tc:
        with (
            tc.tile_pool(name="persist", bufs=1) as pp,
            tc.tile_pool(name="pf", bufs=5) as pf_pool,
            tc.tile_pool(name="pd", bufs=3) as pd_pool,
            tc.tile_pool(name="ot", bufs=8) as ot_pool,
            tc.tile_pool(name="on", bufs=2) as on_pool,
            tc.tile_pool(name="rc", bufs=4) as rc_pool,
            tc.tile_pool(name="yo", bufs=3) as yo_pool,
            tc.tile_pool(name="ps_s", bufs=3, space="PSUM") as ps_s,
            tc.tile_pool(name="ps_p", bufs=1, space="PSUM") as ps_p,
        ):
            xt_sb = pp.tile([128, 4, N], bf)
            wq_sb = pp.tile([128, 4, 128], bf)
            wk_sb = pp.tile([128, 4, 128], bf)
            wv_sb = pp.tile([128, 4, 128], bf)
            wp_sb = pp.tile([128, C], bf)
            tri_sb = pp.tile([128, 128], bf)
            qT = pp.tile([128, N], bf)
            kT = pp.tile([128, N], bf)
            v_sb = pp.tile([128, NKB, 130], bf)

            # Per-head AV accumulators: head h owns one full PSUM bank
            # (4 x 65 slots at [512h : 512h+260]).  matmul start=True clears
            # the has_written bits of the WHOLE 2KB bank (data is untouched),
            # so each head's row places exactly one start=True (kb==0, qq==0)
            # and the other head's bank is never disturbed.
            psPm = ps_p.tile([128, 1024], f32, name="psPm")

            def psO_ap(h, qq):
                return psPm[:, 512 * h + 65 * qq:512 * h + 65 * qq + 65]

            nc.gpsimd.dma_start(out=wq_sb[:, :, :], in_=wq.rearrange("(c p) f -> p c f", p=128))
            nc.gpsimd.dma_start(out=wk_sb[:, :, :], in_=wk.rearrange("(c p) f -> p c f", p=128))
            nc.gpsimd.dma_start(out=wv_sb[:, :, :], in_=wv.rearrange("(c p) f -> p c f", p=128))
            nc.gpsimd.dma_start(out=wp_sb, in_=wp[:, :])
            nc.gpsimd.dma_start(out=tri_sb, in_=tri[:, :])
            nc.vector.memset(v_sb, 1.0)

            xt_re = xt.rearrange("(c p) n -> p c n", p=128)

            exp_counter = [0]

            def emit_exp(dst_i16, src_ps, cols, dve_ok=True):
                """exp(src_ps[:, 0:cols]) -> bf16 bit patterns in dst_i16,
                on ACT normally, on DVE (Schraudolph affine bit-trick) for
                every EXPM-th unit (unless dve_ok is False)."""
                exp_counter[0] += 1
                if dve_ok and EXPM > 0 and exp_counter[0] % EXPM == 0:
                    nc.vector.tensor_scalar(
                        out=dst_i16[:, 0:cols],
                        in0=src_ps[:, 0:cols],
                        scalar1=SCHRAUD_A,
                        scalar2=SCHRAUD_B,
                        op0=Mult,
                        op1=mybir.AluOpType.add,
                    )
                else:
                    nc.scalar.activation(
                        dst_i16.bitcast(bf)[:, 0:cols], src_ps[:, 0:cols], Exp
                    )

            def pa_qk(n, dst, wsb, with_dma):
                def piece():
                    if with_dma:
                        nc.sync.dma_start(
                            out=xt_sb[:, :, QT * n:QT * (n + 1)],
                            in_=xt_re[:, :, QT * n:QT * (n + 1)],
                        )
                    ps = ps_s.tile([128, 1024], f32, tag="s", name=f"pa_{n}")
                    for c in range(4):
                        nc.tensor.matmul(
                            ps[:, 0:512],
                            wsb[:, c, :],
                            xt_sb[:, c, QT * n:QT * (n + 1)],
                            start=(c == 0),
                            stop=(c == 3),
                        )
                    nc.vector.tensor_copy(dst[:, QT * n:QT * (n + 1)], ps[:, 0:512])
                return piece

            def pa_v(kb):
                def piece():
                    ps = ps_s.tile([128, 1024], f32, tag="s", name=f"pav_{kb}")
                    pv = ps[:, 0:128]
                    for c in range(4):
                        nc.tensor.matmul(
                            pv,
                            xt_sb[:, c, KB * kb:KB * (kb + 1)],
                            wv_sb[:, c, :],
                            start=(c == 0),
                            stop=(c == 3),
                        )
                    nc.vector.tensor_copy(
                        v_sb[:, kb, :].rearrange("p (h j) -> p h j", h=2)[:, :, 0:64],
                        pv.rearrange("p (h j) -> p h j", h=2),
                    )
                return piece

            def phase_a_pieces(n):
                return [
                    pa_qk(n, qT, wq_sb, True),
                    pa_qk(n, kT, wk_sb, False),
                    pa_v(4 * n),
                    pa_v(4 * n + 1),
                    pa_v(4 * n + 2),
                    pa_v(4 * n + 3),
                ]

            # diag slot layout keeps every matmul inside one 2KB PSUM bank:
            # r1 -> [0:384], r3 -> [384:512] (bank 0), r2 -> [512:768] (bank 1)
            offs = (0, 512, 384)
            wid = (384, 256, 128)
            # Pd column holding the 128-wide slice for (r, qq):
            # r=1: qq 1,2,3 at 0,128,256 ; r=2: qq 2,3 at 512,640 ; r=3: qq 3 at 384
            diag_col = {(1, 1): 0, (1, 2): 128, (1, 3): 256,
                        (2, 2): 512, (2, 3): 640, (3, 3): 384}

            ot_map = {}
            on_map = {}
            import heapq
            deferred = []  # heap of (due_unit_index, seq, closure)
            seq_counter = [0]

            def defer(due, fn):
                heapq.heappush(deferred, (due, seq_counter[0], fn))
                seq_counter[0] += 1

            def flush(i):
                while deferred and deferred[0][0] <= i:
                    heapq.heappop(deferred)[2]()

            def make_av_full(qt, h, kbs, Pf):
                def av():
                    for j, kb in enumerate(kbs):
                        for qq in range(4):
                            nc.tensor.matmul(
                                psO_ap(h, qq),
                                Pf[:, 512 * j + 128 * qq:512 * j + 128 * (qq + 1)],
                                v_sb[:, kb, 65 * h:65 * h + 65],
                                start=(kb == 0 and qq == 0),
                                stop=(kb == 4 * qt and qq == 0),
                                skip_group_check=True,
                            )
                return av

            def make_av_diag(qt, h, Pd):
                def av():
                    for r in (1, 2, 3):
                        kb = 4 * qt + r
                        for qq in range(r, 4):
                            col = diag_col[(r, qq)]
                            nc.tensor.matmul(
                                psO_ap(h, qq),
                                Pd[:, col:col + 128],
                                v_sb[:, kb, 65 * h:65 * h + 65],
                                start=False,
                                stop=(r == qq),
                                skip_group_check=True,
                            )
                return av

            def make_epilogue(qt, h):
                # Batched normalize: one reciprocal + one multiply over all of
                # this head's slots (strided [65|130] views of psPm), writing
                # the packed [q, d] tile ot_map[qt][:, 128*qq+64*h : +64].
                def epi():
                    if qt not in ot_map:
                        ot_map[qt] = ot_pool.tile(
                            [128, 512], bf, tag="ot", name=f"ot_{qt}"
                        )
                    ot = ot_map[qt].rearrange("p (a b j) -> p a b j", a=4, b=2)
                    rc = rc_pool.tile([128, 4], f32, tag="rc")
                    v4 = psPm[:, 512 * h:512 * h + 260].rearrange(
                        "p (s j) -> p s j", s=4
                    )
                    nc.vector.reciprocal(out=rc, in_=v4[:, :, 64:65])
                    nc.vector.tensor_mul(
                        ot[:, :, h, :],
                        v4[:, :, 0:64],
                        rc.unsqueeze(-1).broadcast_to((128, 4, 64)),
                    )
                return epi

            def make_transposes(qt, qqs):
                def tp():
                    if qt not in on_map:
                        on_map[qt] = on_pool.tile([128, 512], bf, tag="on", name=f"on_{qt}")
                    on = on_map[qt]
                    ot = ot_map[qt]
                    for qq in qqs:
                        nc.sync.dma_start_transpose(
                            on[:, 128 * qq:128 * (qq + 1)],
                            ot[:, 128 * qq:128 * (qq + 1)],
                        )
                return tp

            def make_proj_ob(qt, ob):
                def proj():
                    on = on_map[qt]
                    psYt = ps_s.tile([128, 1024], f32, tag="s", name=f"psY_{qt}_{ob}")
                    psY = psYt[:, 0:512]
                    for qq in range(4):
                        nc.tensor.matmul(
                            psY[:, 128 * qq:128 * (qq + 1)],
                            wp_sb[:, 128 * ob:128 * (ob + 1)],
                            on[:, 128 * qq:128 * (qq + 1)],
                            start=True,
                            stop=True,
                        )
                    y_sb = yo_pool.tile([128, 512], f32, tag="yo")
                    nc.vector.tensor_copy(y_sb, psY)
                    nc.sync.dma_start(
                        out=yt[128 * ob:128 * (ob + 1), QT * qt:QT * (qt + 1)],
                        in_=y_sb,
                    )
                    if ob == 3:
                        on_map.pop(qt)
                        ot_map.pop(qt)
                return proj

            ui = 0
            full_unit_idx = [0]
            pa_qk(0, qT, wq_sb, True)()
            pa_qk(0, kT, wk_sb, False)()
            pa_pending = [pa_v(0), pa_v(1), pa_v(2), pa_v(3)]
            for qt in range(NQT):
                for piece in pa_pending:
                    piece()
                pa_pending = phase_a_pieces(qt + 1) if qt + 1 < NQT else []
                nfull = 4 * qt + 1
                d_av = 2 if nfull <= 4 else 3
                for h in range(2):
                    b0 = 64 * h
                    # ---- full units: kb groups of 2 over kb = 0..4qt
                    kb = 0
                    while kb < nfull:
                        w = min(2, nfull - kb)
                        kbs = list(range(kb, kb + w))
                        psS = ps_s.tile([128, 1024], f32, tag="s")
                        for j, kbj in enumerate(kbs):
                            nc.tensor.matmul(
                                psS[:, 512 * j:512 * (j + 1)],
                                kT[b0:b0 + 64, KB * kbj:KB * (kbj + 1)],
                                qT[b0:b0 + 64, QT * qt:QT * (qt + 1)],
                                start=True,
                                stop=True,
                            )
                        pfi = pf_pool.tile([128, 1024], i16, tag="pf")
                        emit_exp(pfi, psS, 512 * w, dve_ok=(kb + w < nfull - 2))
                        Pf = pfi.bitcast(bf)
                        if kbs[-1] == 4 * qt:
                            j = w - 1
                            nc.gpsimd.tensor_mul(
                                Pf[:, 512 * j:512 * j + 128],
                                Pf[:, 512 * j:512 * j + 128],
                                tri_sb,
                            )
                        flush(ui)
                        defer(ui + d_av, make_av_full(qt, h, kbs, Pf))
                        if pa_pending:
                            pa_pending.pop(0)()
                        ui += 1
                        kb += w
                    # ---- diag unit: r = 1..3 packed [r1|r3|r2]
                    psD = ps_s.tile([128, 1024], f32, tag="s")
                    for r in (1, 2, 3):
                        kbr = 4 * qt + r
                        nc.tensor.matmul(
                            psD[:, offs[r - 1]:offs[r - 1] + wid[r - 1]],
                            kT[b0:b0 + 64, KB * kbr:KB * (kbr + 1)],
                            qT[b0:b0 + 64, QT * qt + 128 * r:QT * qt + 128 * r + wid[r - 1]],
                            start=True,
                            stop=True,
                        )
                    Pdi = pd_pool.tile([128, 768], i16, tag="pd")
                    emit_exp(Pdi, psD, 768, dve_ok=False)
                    Pd = Pdi.bitcast(bf)
                    for r in (1, 2, 3):
                        nc.gpsimd.tensor_mul(
                            Pd[:, offs[r - 1]:offs[r - 1] + 128],
                            Pd[:, offs[r - 1]:offs[r - 1] + 128],
                            tri_sb,
                        )
                    flush(ui)
                    defer(ui + d_av, make_av_diag(qt, h, Pd))
                    defer(ui + d_av + 1, make_epilogue(qt, h))
                    if h == 1:
                        defer(ui + d_av + 2, make_transposes(qt, (0, 1)))
                        defer(ui + d_av + 3, make_transposes(qt, (2, 3)))
                        for ob in range(4):
                            defer(ui + d_av + 4 + ob, make_proj_ob(qt, ob))
                    if pa_pending:
                        pa_pending.pop(0)()
                    ui += 1
            flush(10 ** 9)

    nc.compile()
    return nc


def kernel(x, w_qkv, w_proj, b_proj):
    global LAST_RESULTS
    from concourse.bass_utils import run_bass_kernel_spmd

    if "nc" not in _CACHE:
        _CACHE["nc"] = _build()
    nc = _CACHE["nc"]

    x = np.asarray(x)
    w_qkv = np.asarray(w_qkv)
    w_proj = np.asarray(w_proj)
    b_proj = np.asarray(b_proj)
    bf16 = ml_dtypes.bfloat16
    scale = D ** -0.5

    tri = np.triu(np.ones((128, 128), np.float32)).astype(bf16)
    in_maps = []
    for core in range(8):
        b, g = divmod(core, 4)
        xt = np.ascontiguousarray(x[b].T).astype(bf16)
        wq = np.ascontiguousarray((w_qkv[128 * g:128 * (g + 1), :].T * scale)).astype(bf16)
        wk = np.ascontiguousarray(w_qkv[C + 128 * g:C + 128 * (g + 1), :].T).astype(bf16)
        wv = np.ascontiguousarray(w_qkv[2 * C + 128 * g:2 * C + 128 * (g + 1), :].T).astype(bf16)
        wp = np.ascontiguousarray(w_proj[:, 128 * g:128 * (g + 1)].T).astype(bf16)
        in_maps.append({"xt": xt, "wq": wq, "wk": wk, "wv": wv, "wp": wp,
                        "tri": tri})

    res = run_bass_kernel_spmd(
        nc,
        in_maps,
        core_ids=list(range(8)),
        trace=bool(os.environ.get("KERNEL_TRACE")),
    )
    LAST_RESULTS = res

    y = np.empty((B, N, C), np.float32)
    for b in range(B):
        acc = res.results[4 * b]["yt"].astype(np.float32)
        for g in range(1, 4):
            acc = acc + res.results[4 * b + g]["yt"]
        y[b] = acc.T + b_proj
    return y
